# revision 1
# baseline (speedup 1.0000x reference)
"""CatNet spiking CNN on 8x TRN2 NeuronCores (data-parallel over batch N=64).

Integrated spike-count formulation: for an IF neuron (soft reset, thresh 1)
with cumulative input P_t and cumulative spike count S_t:
    s_t = (P_t - S_{t-1} >= 1 - (t+1)*bias),   S_t = S_{t-1} + s_t.
Convs are linear, so the cumulative drive of a layer fed by spikes is
conv(S^prev_t): each timestep computes F = conv(S^prev_t) fresh on the PE
(no cross-t PSUM accumulation), then one fused DVE compare
(scalar_tensor_tensor) + one DVE add per layer.

States are small integers -> exact in bf16. Matmuls are bf16 with hi/lo
weight splits (~fp32-class precision, 1 cycle/column). The 2x2 "pool" (x1.1)
runs as identity matmuls with the 1.1 folded into the h3 compare (op0=mult)
and into wf1 (host-scaled); fc1's (Q-1) shift is folded into the f1
threshold via rowsum(wf1').
"""
import numpy as np
import ml_dtypes

import concourse.bass as bass
import concourse.mybir as mybir
from concourse import tile
from concourse.bass_utils import run_bass_kernel_spmd
from concourse.tile_rust import add_dep_helper

F32 = mybir.dt.float32
BF16 = mybir.dt.bfloat16
BF = ml_dtypes.bfloat16
ALU = mybir.AluOpType
ACTF = mybir.ActivationFunctionType

NCORE = 8
NI = 8
T = 16
WP = 32                 # padded row stride (30 cols used; 64B-aligned bf16 rows)
HPAD = 30
PADPIX = HPAD * WP      # 960
NPIX = 784
NH = 392
TAPS = [(ky, kx) for ky in range(3) for kx in range(3)]


def _split_excess_waits(nc, maxw=1):
    """This walrus accepts only one sync-wait per instruction; hoist extras
    onto preceding same-engine nops."""
    ctr = [0]
    for bb in nc.m.functions[0].blocks:
        nl = []
        for inst in bb.instructions:
            w = list(inst.sync_info.on_wait) if inst.sync_info else []
            if len(w) > maxw:
                keep, exc = w[:maxw], w[maxw:]
                for cs in range(0, len(exc), maxw):
                    nop = mybir.InstNoOp(name=f"I-ws-{ctr[0]}", ins=[], outs=[])
                    ctr[0] += 1
                    nop.engine = inst.engine
                    nop.sync_info = mybir.SyncInfo(
                        on_wait=list(exc[cs:cs + maxw]), on_update=[])
                    nc.register_instruction(nop)
                    nl.append(nop)
                inst.sync_info = mybir.SyncInfo(
                    on_wait=list(keep), on_update=list(inst.sync_info.on_update))
            nl.append(inst)
        bb.instructions[:] = nl


def _rap(handle, offset, dims):
    """Raw access pattern on a DRAM tensor handle."""
    return bass.AP(handle, offset, [list(d) for d in dims])


def _pad3(ap_pad, nrow=128):
    """[P, 960] padded tile -> [P, 30, 32] view."""
    return ap_pad.rearrange("p (h w) -> p h w", w=WP)


def _interior(ap_pad):
    """[P, 960] padded tile -> [P, 28, 28] interior view."""
    return _pad3(ap_pad)[:, 1:29, 1:29]


def _cwin(ap_pad, ky, kx, h0):
    """Conv-tap window: out rows h0..h0+13 -> padded rows h0+ky.., cols kx.."""
    return _pad3(ap_pad)[:, h0 + ky:h0 + ky + 14, kx:kx + 28]


def _pwin(ap_pad, e0, e1, dy, dx):
    """Pool window on partitions e0:e1: [P, 14, 14], elem (r,c) = padded
    (2r+1+dy, 2c+1+dx)."""
    v = ap_pad[e0:e1, :].rearrange("p (h w) -> p h w", w=WP)
    return v[:, 1 + dy:1 + dy + 28:2, 1 + dx:1 + dx + 28:2]


def build_nc(nt=T, ni=NI):
    nc = bass.Bass()
    qn = ni // 4
    pn = ni // 2

    xst = nc.dram_tensor("xst", [ni * nt, 27, 2, NH], BF16, kind="ExternalInput")
    w1q = nc.dram_tensor("w1q", [128, 3, 128], BF16, kind="ExternalInput")
    w2q = nc.dram_tensor("w2q", [128, 9, 2, 64], BF16, kind="ExternalInput")
    w3q = nc.dram_tensor("w3q", [128, 9, 2, 64], BF16, kind="ExternalInput")
    idq = nc.dram_tensor("idq", [128, 64], BF16, kind="ExternalInput")
    wf1s = nc.dram_tensor("wf1s", [98, 2, 128, 128], BF16, kind="ExternalInput")
    wf2s = nc.dram_tensor("wf2s", [128, 2, 10], BF16, kind="ExternalInput")
    thrL1 = nc.dram_tensor("thrL1", [128, nt], F32, kind="ExternalInput")
    thrH1 = nc.dram_tensor("thrH1", [128, nt], F32, kind="ExternalInput")
    thrH2 = nc.dram_tensor("thrH2", [128, nt], F32, kind="ExternalInput")
    thrF1 = nc.dram_tensor("thrF1", [128, nt], F32, kind="ExternalInput")
    bf2t = nc.dram_tensor("bf2t", [10, 1], F32, kind="ExternalInput")
    out = nc.dram_tensor("out", [ni, 10], F32, kind="ExternalOutput")
    sh3d = nc.dram_tensor("sh3d", [64, 196, ni, nt], BF16, kind="Internal")

    with tile.TileContext(nc) as tc:
        with (
            tc.tile_pool(name="wpool", bufs=1) as wp,
            tc.tile_pool(name="state", bufs=1) as stp,
            tc.tile_pool(name="scratch", bufs=1) as scp,
            tc.tile_pool(name="psum", bufs=1, space="PSUM") as psp,
        ):
            # ---- weights / thresholds ----
            w1s = wp.tile([128, 3, 128], BF16, name="w1s")
            nc.sync.dma_start(w1s[:, :, :], w1q[:, :, :])
            w2s = wp.tile([128, 9, 2, 64], BF16, name="w2s")
            nc.sync.dma_start(w2s[:, :, :, :], w2q[:, :, :, :])
            w3s = wp.tile([128, 9, 2, 64], BF16, name="w3s")
            nc.sync.dma_start(w3s[:, :, :, :], w3q[:, :, :, :])
            ids = wp.tile([128, 64], BF16, name="ids")
            nc.sync.dma_start(ids[:, :], idq[:, :])
            wf2l = wp.tile([128, 2, 10], BF16, name="wf2l")
            nc.sync.dma_start(wf2l[:, :, :], wf2s[:, :, :])
            tL1 = wp.tile([128, nt], F32, name="tL1")
            nc.sync.dma_start(tL1[:, :], thrL1[:, :])
            tH1 = wp.tile([128, nt], F32, name="tH1")
            nc.sync.dma_start(tH1[:, :], thrH1[:, :])
            tH2 = wp.tile([128, nt], F32, name="tH2")
            nc.sync.dma_start(tH2[:, :], thrH2[:, :])
            tF1 = wp.tile([128, nt], F32, name="tF1")
            nc.sync.dma_start(tF1[:, :], thrF1[:, :])
            bf2l = wp.tile([10, 1], F32, name="bf2l")
            nc.sync.dma_start(bf2l[:, :], bf2t[:, :])

            # ---- persistent states ----
            sL1 = []        # per image [128, 784]: rows 0-63 S_x1, 64-95 S_h0
            for i in range(ni):
                st = stp.tile([128, NPIX], BF16, name=f"sL1_{i}")
                nc.gpsimd.memset(st[:, :], 0.0)
                sL1.append(st)
            sH0q = []       # per quad, padded, img g at rows 32g (conv2 rhs)
            for q in range(qn):
                st = stp.tile([128, PADPIX], BF16, name=f"sH0q_{q}")
                nc.gpsimd.memset(st[:, :], 0.0)
                sH0q.append(st)
            sH1, sH2, qH3 = [], [], []
            for p in range(pn):
                s1 = stp.tile([128, PADPIX], BF16, name=f"sH1_{p}")
                nc.gpsimd.memset(s1[:, :], 0.0)
                sH1.append(s1)
                s2 = stp.tile([128, PADPIX], BF16, name=f"sH2_{p}")
                nc.gpsimd.memset(s2[:, :], 0.0)
                sH2.append(s2)
                q3 = stp.tile([128, 196, nt + 1], BF16, name=f"qH3_{p}")
                nc.gpsimd.memset(q3[:, :, :], 1.0)   # Q = S + 1
                qH3.append(q3)

            last_in_bank = {}  # bank-key -> last matmul inst of prior chain

            def chain_dep(key, first_mm):
                if key in last_in_bank:
                    add_dep_helper(first_mm.ins, last_in_bank[key].ins,
                                   sync=False, reason="psum group order")

            FdL = {}
            Fd1 = {}
            Fd2 = {}
            Fd3 = {}

            def emit_wave(chains):
                n = max(len(c) for c in chains)
                for j in range(n):
                    for c in chains:
                        if j < len(c):
                            fw, lhs, rhs, st, sp, tp, key = c[j]
                            mm = nc.tensor.matmul(fw, lhs, rhs, start=st,
                                                  stop=sp, tile_position=tp)
                            if st:
                                chain_dep(key, mm)
                            if sp:
                                last_in_bank[key] = mm

            def stage_l1(t, q):
                imgs = [4 * q + g for g in range(4)]
                # ---------- L1 ----------
                imst = scp.tile([128, NPIX], BF16, tag="imst",
                                name=f"imst_{t}_{q}", bufs=3)
                for g, i in enumerate(imgs):
                    off = (i * nt + t) * 27 * 2 * NH
                    nc.sync.dma_start(imst[32 * g:32 * g + 27, :],
                                      _rap(xst, off, [[2 * NH, 27], [1, 2 * NH]]))
                fLs = FdL
                l1_mms = {i: [] for i in imgs}
                for g, i in enumerate(imgs):
                    for h in range(2):
                        fLs[(i, h)] = psp.tile([128, 512], F32, tag="F",
                                               name=f"fL_{t}_{i}_{h}", bufs=6)
                        fw = fLs[(i, h)][:, 0:NH]
                        rh = imst[32 * g:32 * g + 27, NH * h:NH * h + NH]
                        for v in range(3):
                            l1_mms[i].append(
                                (fw, w1s[32 * g:32 * g + 27, v, :], rh,
                                 v == 0, v == 2, (32 * g, 0), ("L", i, h)))
                # round-robin across images for row-group diversity
                for j in range(6):
                    for g, i in enumerate(imgs):
                        fw, lhs, rh, st, sp, tp, key = l1_mms[i][j]
                        mm = nc.tensor.matmul(fw, lhs, rh, start=st, stop=sp,
                                              tile_position=tp)
                        if st:
                            chain_dep(key, mm)
                        if sp:
                            last_in_bank[key] = mm
                for g, i in enumerate(imgs):
                    sL = scp.tile([128, NPIX], BF16, tag="s",
                                  name=f"s_{t}_{i}", bufs=6)
                    for h in range(2):
                        nc.vector.scalar_tensor_tensor(
                            sL[:, NH * h:NH * h + NH],
                            fLs[(i, h)][:, 0:NH],
                            tL1[:, t:t + 1],
                            sL1[i][:, NH * h:NH * h + NH],
                            op0=ALU.subtract, op1=ALU.is_ge)
                    nc.vector.tensor_tensor(
                        sL1[i][:, :], sL1[i][:, :], sL[:, :], ALU.add)
                    # maintain padded quad h0 copy on ScalarE (partition shift)
                    nc.scalar.copy(
                        _interior(sH0q[q][32 * g:32 * g + 32, :]),
                        sL1[i][64:96, :].rearrange("p (h w) -> p h w", w=28))

            def stage_h1_mm(t, q):
                imgs = [4 * q + g for g in range(4)]
                # ---------- h1: conv2(S_h0) + I*S_x1 ----------
                f1s = Fd1
                for pp in range(2):
                    p = 2 * q + pp
                    for h in range(2):
                        f1s[(p, h)] = psp.tile([128, 512], F32, tag="F",
                                               name=f"f1_{t}_{p}_{h}", bufs=6)

                def h1_chain_list(pp, h, e):
                    p = 2 * q + pp
                    i = 4 * q + 2 * pp + e
                    g = 2 * pp + e
                    col = 64 * e
                    fw = f1s[(p, h)][col:col + 64, 0:NH]
                    key = ("h1", p, h)
                    mms = [(fw, ids[0:64, :],
                            sL1[i][0:64, :].rearrange("p (h w) -> p h w", w=28)[
                                :, 14 * h:14 * h + 14, :],
                            True, False, (0, col), key)]
                    for k, (ky, kx) in enumerate(TAPS):
                        rhs = _cwin(sH0q[q][32 * g:32 * g + 32, :], ky, kx, 14 * h)
                        for hl in range(2):
                            mms.append((fw, w2s[32 * g:32 * g + 32, k, hl, :], rhs,
                                        False, (k == 8 and hl == 1),
                                        (32 * g, col), key))
                    return mms

                waves = ([h1_chain_list(0, 0, 0), h1_chain_list(0, 1, 1),
                          h1_chain_list(1, 0, 0), h1_chain_list(1, 1, 1)],
                         [h1_chain_list(0, 0, 1), h1_chain_list(0, 1, 0),
                          h1_chain_list(1, 0, 1), h1_chain_list(1, 1, 0)])

                return waves

            def stage_h1_post(t, q):
                f1s = Fd1
                for pp in range(2):
                    p = 2 * q + pp
                    sP = scp.tile([128, NPIX], BF16, tag="s",
                                  name=f"sh1_{t}_{p}", bufs=6)
                    for h in range(2):
                        nc.vector.scalar_tensor_tensor(
                            sP[:, NH * h:NH * h + NH].rearrange(
                                "p (a b) -> p a b", b=28),
                            f1s[(p, h)][:, 0:NH].rearrange(
                                "p (a b) -> p a b", b=28),
                            tH1[:, t:t + 1],
                            _pad3(sH1[p][:, :])[:, 1 + 14 * h:15 + 14 * h, 1:29],
                            op0=ALU.subtract, op1=ALU.is_ge)
                    nc.vector.tensor_tensor(
                        _interior(sH1[p][:, :]), _interior(sH1[p][:, :]),
                        sP[:, :].rearrange("p (h w) -> p h w", w=28), ALU.add)

            def stage_h2_mm(t, q):
                imgs = [4 * q + g for g in range(4)]
                # ---------- h2: conv3(S_h1) ----------
                f2s = Fd2
                for pp in range(2):
                    p = 2 * q + pp
                    for h in range(2):
                        f2s[(p, h)] = psp.tile([128, 512], F32, tag="F",
                                               name=f"f2_{t}_{p}_{h}", bufs=6)

                def h2_chain_list(pp, h, e):
                    p = 2 * q + pp
                    row = 64 * e
                    col = 64 * e
                    fw = f2s[(p, h)][col:col + 64, 0:NH]
                    key = ("h2", p, h)
                    mms = []
                    first = True
                    for k, (ky, kx) in enumerate(TAPS):
                        rhs = _cwin(sH1[p][row:row + 64, :], ky, kx, 14 * h)
                        for hl in range(2):
                            mms.append((fw, w3s[row:row + 64, k, hl, :], rhs,
                                        first, (k == 8 and hl == 1),
                                        (row, col), key))
                            first = False
                    return mms

                waves = ([h2_chain_list(0, 0, 0), h2_chain_list(0, 1, 1),
                          h2_chain_list(1, 0, 0), h2_chain_list(1, 1, 1)],
                         [h2_chain_list(0, 0, 1), h2_chain_list(0, 1, 0),
                          h2_chain_list(1, 0, 1), h2_chain_list(1, 1, 0)])

                return waves

            def stage_h2_post(t, q):
                f2s = Fd2
                for pp in range(2):
                    p = 2 * q + pp
                    sP = scp.tile([128, NPIX], BF16, tag="s",
                                  name=f"sh2_{t}_{p}", bufs=6)
                    for h in range(2):
                        nc.vector.scalar_tensor_tensor(
                            sP[:, NH * h:NH * h + NH].rearrange(
                                "p (a b) -> p a b", b=28),
                            f2s[(p, h)][:, 0:NH].rearrange(
                                "p (a b) -> p a b", b=28),
                            tH2[:, t:t + 1],
                            _pad3(sH2[p][:, :])[:, 1 + 14 * h:15 + 14 * h, 1:29],
                            op0=ALU.subtract, op1=ALU.is_ge)
                    nc.vector.tensor_tensor(
                        _interior(sH2[p][:, :]), _interior(sH2[p][:, :]),
                        sP[:, :].rearrange("p (h w) -> p h w", w=28), ALU.add)

            def stage_h3_mm(t, q):
                imgs = [4 * q + g for g in range(4)]
                # ---------- h3: 2x2 sum pool (x1.1 in compare) ----------
                f3s = Fd3
                for pp in range(2):
                    p = 2 * q + pp
                    f3s[p] = psp.tile([128, 512], F32, tag="F3",
                                      name=f"f3_{t}_{p}", bufs=2)

                def pool_chain_list(pp, e):
                    p = 2 * q + pp
                    col = 64 * e
                    fw = f3s[p][col:col + 64, 0:196]
                    key = ("h3", p)
                    mms = []
                    for wi, (dy, dx) in enumerate(
                            ((0, 0), (0, 1), (1, 0), (1, 1))):
                        rhs = _pwin(sH2[p][:, :], 64 * e, 64 * e + 64, dy, dx)
                        mms.append((fw, ids[64 * e:64 * e + 64, :], rhs,
                                    wi == 0, wi == 3, (64 * e, col), key))
                    return mms

                waves = ([pool_chain_list(0, 0), pool_chain_list(1, 1)],
                         [pool_chain_list(0, 1), pool_chain_list(1, 0)])

                return waves

            def stage_h3_post(t, q):
                f3s = Fd3
                for pp in range(2):
                    p = 2 * q + pp
                    sP = scp.tile([128, NPIX], BF16, tag="s",
                                  name=f"sh3_{t}_{p}", bufs=6)
                    nc.vector.scalar_tensor_tensor(
                        sP[:, 0:196], f3s[p][:, 0:196], 1.1,
                        qH3[p][:, :, t], op0=ALU.mult, op1=ALU.is_ge)
                    nc.vector.tensor_tensor(
                        qH3[p][:, :, t + 1], qH3[p][:, :, t],
                        sP[:, 0:196], ALU.add)

            # ================= time loop (stage-major, cross-stage waves) ====
            for t in range(nt):
                for q in range(qn):
                    stage_l1(t, q)
                if qn == 2:
                    w1a_, w1b_ = stage_h1_mm(t, 0)
                    emit_wave(w1a_)
                    emit_wave(w1b_)
                    stage_h1_post(t, 0)
                    h2a, h2b = stage_h2_mm(t, 0)
                    h1a, h1b = stage_h1_mm(t, 1)
                    emit_wave(h2a + h1a)
                    emit_wave(h2b + h1b)
                    stage_h1_post(t, 1)
                    stage_h2_post(t, 0)
                    h3a, h3b = stage_h3_mm(t, 0)
                    h2a, h2b = stage_h2_mm(t, 1)
                    emit_wave(h3a + h2a)
                    emit_wave(h3b + h2b)
                    stage_h2_post(t, 1)
                    stage_h3_post(t, 0)
                    pa, pb = stage_h3_mm(t, 1)
                    emit_wave(pa)
                    emit_wave(pb)
                    stage_h3_post(t, 1)
                else:
                    for q in range(qn):
                        for wv in stage_h1_mm(t, q):
                            emit_wave(wv)
                        stage_h1_post(t, q)
                        for wv in stage_h2_mm(t, q):
                            emit_wave(wv)
                        stage_h2_post(t, q)
                        for wv in stage_h3_mm(t, q):
                            emit_wave(wv)
                        stage_h3_post(t, q)

            # ---- ship Q trajectories to DRAM ----
            for p in range(pn):
                for e in range(2):
                    i = 2 * p + e
                    dst = _rap(sh3d, i * nt,
                               [[196 * ni * nt, 64], [ni * nt, 196], [1, nt]])
                    nc.sync.dma_start(dst, qH3[p][64 * e:64 * e + 64, :, 1:nt + 1])

            # ---------- fc1 ----------
            ff1 = psp.tile([128, 512], F32, tag="F3", name="ff1", bufs=2)
            for px in range(98):
                wt = scp.tile([128, 2, 128], BF16, tag="wf1t",
                              name=f"wf1t_{px}", bufs=4)
                nc.sync.dma_start(wt[:, :, :], wf1s[px, :, :, :].rearrange("a p m -> p a m"))
                rt = scp.tile([128, ni * nt], BF16, tag="rf1t",
                              name=f"rf1t_{px}", bufs=4)
                src = _rap(sh3d, (2 * px) * ni * nt,
                           [[ni * nt, 2], [196 * ni * nt, 64], [1, ni * nt]])
                nc.sync.dma_start(rt[:, :], src)
                for hl in range(2):
                    nc.tensor.matmul(ff1[:, 0:ni * nt], wt[:, hl, :], rt[:, :],
                                     start=(px == 0 and hl == 0),
                                     stop=(px == 97 and hl == 1))

            # f1 spike scan (Q_h3 includes +1 offset; folded into thrF1)
            sf1 = stp.tile([128, ni], BF16, name="sf1")
            nc.gpsimd.memset(sf1[:, :], 0.0)
            for t in range(nt):
                sPf = scp.tile([128, ni], BF16, tag="sf", name=f"sf_{t}", bufs=2)
                nc.vector.scalar_tensor_tensor(
                    sPf[:, :],
                    ff1[:, 0:ni * nt].rearrange("p (n t) -> p n t", t=nt)[:, :, t],
                    tF1[:, t:t + 1], sf1[:, :],
                    op0=ALU.subtract, op1=ALU.is_ge)
                nc.vector.tensor_tensor(sf1[:, :], sf1[:, :], sPf[:, :], ALU.add)

            # ---------- fc2 + readout ----------
            ff2 = psp.tile([128, 512], F32, tag="F3", name="ff2", bufs=2)
            for hl in range(2):
                nc.tensor.matmul(ff2[0:10, 0:ni], wf2l[:, hl, :], sf1[:, :],
                                 start=(hl == 0), stop=(hl == 1))
            osb = scp.tile([10, ni], F32, tag="osb", name="osb")
            nc.scalar.activation(osb[:, :], ff2[0:10, 0:ni], ACTF.Identity,
                                 bias=bf2l[:, :], scale=1.0 / nt)
            nc.sync.dma_start(out[:, :].rearrange("n o -> o n"), osb[:, :])

    _split_excess_waits(nc)
    return nc


# ---------------- host side ----------------

def _split(a):
    hi = np.asarray(a, np.float64).astype(BF)
    lo = (np.asarray(a, np.float64) - hi.astype(np.float64)).astype(BF)
    return hi, lo


def _split3(a):
    a = np.asarray(a, np.float64)
    p0 = a.astype(BF)
    r = a - p0.astype(np.float64)
    p1 = r.astype(BF)
    p2 = (r - p1.astype(np.float64)).astype(BF)
    return p0, p1, p2


def _prep_shared(w1a, b1a, w1, b1, w2, b2, w3, b3, wf1, bf1, wf2, bf2, nt=T):
    d = {}
    # L1 lhsT [9, 128]: cols 0-63 w1a, 64-95 w1, 96-127 zero; rows k=3ky+kx
    # 3-term split W = W0+W1+W2; K=27 stacked variants to cover products with
    # the 3-term x split (X0,X1,X2): [W0;W1;W2], [W1;W0;0], [W2;0;W0]
    l1 = np.zeros((9, 128), np.float64)
    l1[:, 0:64] = w1a.reshape(64, 9).T
    l1[:, 64:96] = w1.reshape(32, 9).T
    W0, W1, W2 = _split3(l1)
    Z = np.zeros_like(W0)
    variants = [np.concatenate([W0, W1, W2]), np.concatenate([W1, W0, Z]),
                np.concatenate([W2, Z, W0])]
    w1qa = np.zeros((128, 3, 128), BF)
    for g in range(4):
        for v in range(3):
            w1qa[32 * g:32 * g + 27, v, :] = variants[v]
    d["w1q"] = w1qa
    # conv2 [tap, c, m] replicated at 4 bases
    a2 = np.transpose(w2.reshape(64, 32, 3, 3), (2, 3, 1, 0)).reshape(9, 32, 64)
    h2, l2 = _split(a2)
    w2qa = np.zeros((128, 9, 2, 64), BF)
    for g in range(4):
        w2qa[32 * g:32 * g + 32, :, 0, :] = np.transpose(h2, (1, 0, 2))
        w2qa[32 * g:32 * g + 32, :, 1, :] = np.transpose(l2, (1, 0, 2))
    d["w2q"] = w2qa
    a3 = np.transpose(w3.reshape(64, 64, 3, 3), (2, 3, 1, 0)).reshape(9, 64, 64)
    h3v, l3v = _split(a3)
    w3qa = np.zeros((128, 9, 2, 64), BF)
    for e in range(2):
        w3qa[64 * e:64 * e + 64, :, 0, :] = np.transpose(h3v, (1, 0, 2))
        w3qa[64 * e:64 * e + 64, :, 1, :] = np.transpose(l3v, (1, 0, 2))
    d["w3q"] = w3qa
    idqa = np.zeros((128, 64), BF)
    for e in range(2):
        idqa[64 * e:64 * e + 64, :] = np.eye(64, dtype=BF)
    d["idq"] = idqa
    # fc1 tiles [pxh, hl, row=64j+c, m]; fc1 consumes h3 spikes (no 1.1)
    wf1p = np.asarray(wf1, np.float64)                  # [128, 64, 14, 14]
    wf1f = wf1p.reshape(128, 64, 196)                   # px = 14*h + w
    wf1sa = np.zeros((98, 2, 128, 128), BF)
    hi1, lo1 = _split(wf1f)
    for pxh in range(98):
        for j in range(2):
            px = 2 * pxh + j
            wf1sa[pxh, 0, 64 * j:64 * j + 64, :] = hi1[:, :, px].T
            wf1sa[pxh, 1, 64 * j:64 * j + 64, :] = lo1[:, :, px].T
    d["wf1s"] = wf1sa
    h2v, l2v = _split(np.asarray(wf2, np.float64).T)    # [128, 10]
    wf2sa = np.zeros((128, 2, 10), BF)
    wf2sa[:, 0, :] = h2v
    wf2sa[:, 1, :] = l2v
    d["wf2s"] = wf2sa
    # thresholds [128, nt]
    tsteps = np.arange(1, nt + 1)
    thrL1 = np.ones((128, nt), np.float32)
    thrL1[0:64] = 1.0 - tsteps[None, :] * np.asarray(b1a, np.float64)[:, None]
    thrL1[64:96] = 1.0 - tsteps[None, :] * np.asarray(b1, np.float64)[:, None]
    d["thrL1"] = thrL1
    thrH1 = np.ones((128, nt), np.float32)
    b2d = np.asarray(b2, np.float64)
    thrH1[0:64] = 1.0 - tsteps[None, :] * b2d[:, None]
    thrH1[64:128] = 1.0 - tsteps[None, :] * b2d[:, None]
    d["thrH1"] = thrH1
    thrH2 = np.ones((128, nt), np.float32)
    b3d = np.asarray(b3, np.float64)
    thrH2[0:64] = 1.0 - tsteps[None, :] * b3d[:, None]
    thrH2[64:128] = 1.0 - tsteps[None, :] * b3d[:, None]
    d["thrH2"] = thrH2
    # f1: (F_comp = wf1p . Q) >= S + 1 + rowsum(wf1p) - (t+1)*bf1
    rs = wf1f.sum(axis=(1, 2))                          # [128]
    thrF1 = np.zeros((128, nt), np.float32)
    thrF1[:, :] = (1.0 + rs[:, None]
                   - tsteps[None, :] * np.asarray(bf1, np.float64)[:, None])
    d["thrF1"] = thrF1
    d["bf2t"] = np.asarray(bf2, np.float32).reshape(10, 1)
    return d


def _prep_x(xc, nt=T):
    """xc [ni, 1, 28, 28, nt] -> host im2col of the 3-term-split cumulative
    input, stacked [ni*nt, 27, 2, 392] = ([X0;X1;X2], half, px)."""
    ni = xc.shape[0]
    X = np.cumsum(np.asarray(xc, np.float64), axis=-1)[:, 0]   # [ni, 28, 28, nt]
    X = np.moveaxis(X, -1, 1)                                  # [ni, nt, 28, 28]
    pad = np.zeros((ni * nt, 30, 30), np.float64)
    pad[:, 1:29, 1:29] = X.reshape(ni * nt, 28, 28)
    col = np.zeros((ni * nt, 9, 2, NH), np.float64)
    for k, (ky, kx) in enumerate(TAPS):
        for h in range(2):
            col[:, k, h, :] = pad[:, 14 * h + ky:14 * h + ky + 14,
                                  kx:kx + 28].reshape(ni * nt, NH)
    x0, x1, x2 = _split3(col)
    return np.concatenate([x0, x1, x2], axis=1)  # [ni*nt, 27, 2, NH]


_NC_CACHE = {}


def kernel(x, w1a, b1a, w1, b1, w2, b2, w3, b3, wf1, bf1, wf2, bf2):
    x = np.asarray(x)
    n_total = x.shape[0]
    ni = n_total // NCORE
    key = (ni, T)
    if key not in _NC_CACHE:
        _NC_CACHE[key] = build_nc(T, ni)
    nc = _NC_CACHE[key]

    shared = _prep_shared(w1a, b1a, w1, b1, w2, b2, w3, b3, wf1, bf1, wf2, bf2)
    in_maps = []
    for c in range(NCORE):
        m = dict(shared)
        m["xst"] = _prep_x(x[c * ni:(c + 1) * ni])
        in_maps.append(m)

    res = run_bass_kernel_spmd(nc, in_maps, list(range(NCORE))).results
    return np.concatenate([res[c]["out"] for c in range(NCORE)], axis=0)

